# revision 34
# baseline (speedup 1.0000x reference)
"""Trainium2 Bass kernel for the Actor MLP scorer (gnn_message_passing).

Computation (see reference):
    node_e  = node_embeddings[action_nodes]          # [A, 128] gather
    feats   = [node_e | region_embeddings[action_regions] | const_tail]   # [A, 1427]
    h1..h3  = relu MLP (256 wide), logits = h3 @ W4 + b4                  # [A]
    probs   = softmax(logits) over ALL actions

Strategy (8 NeuronCores, data-parallel over actions):
  - Shard A=100000 actions as 12500/core.  Per core, actions are sorted by
    node-id bucket (< 32768 vs >= 32768) so the node-embedding gather can use
    the int16-indexed DMA-gather ucode with two base-offset views of a bf16
    copy of the table.  Groups are padded to the static capacities C0/C1; a
    mask input removes pad slots from the softmax.  Outputs un-permuted on
    host.
  - A dep-free 128-idx dummy gather pays the ~9us Q7 gather-ucode load at
    t~=9us.  All 16 sweep-aligned dma_gather calls are then emitted upfront,
    round-robin over the 4 SWDGE queues; the first two calls are small (256
    idxs) with dedicated 4KB index tiles so the pipeline's first data lands
    ~21us.  PE transposes + copies into the [dim, action] activation layout
    are emitted inside the sweep loop so the PE FIFO never waits on a late
    chunk.  (dma_gather(transpose=True) corrupts data when multiple queues
    run concurrently, and HWDGE dma-transpose is serialized by the framework
    against the SWDGE gathers — both measured dead ends; PE transposes
    pipeline at 55ns/128-block and are cheap.)
  - Layer 1 is decomposed: feats @ W1 = node_e @ W1[:128]
        + onehot(region) @ (region_embeddings @ W1[128:256])
        + tail @ W1[256:]  (host-folded into the relu bias b1c).
  - Weights are pre-cast to bf16 on host (no SWDGE cast-DMAs on the Pool
    queue); constant projections are host-folded.  ~190 dummy matmuls warm
    the PE HAM clock gate so real matmuls run at 2.4 GHz from the start;
    graded sweep sizes (2,2,4,4,...) keep the PE busy while the first
    gathers land, avoiding a mid-ramp clock drop.
  - fp8/DoubleRow was evaluated and rejected: e4m3 weights+activations give
    0.11-0.25 logit rel err vs the 2e-2 budget (bf16 chain is 9.1e-3).
  - Softmax: logits row staged to HBM in two mid-loop preps, reloaded as
    [128, c] slices and exp(l-4)'d; the final 1024 logits are transposed on
    the PE (rank-1 matmuls) to skip the HBM round-trip on the critical tail.
    Partition reduce/broadcast via tiny PE matmuls; one [1,1] AllReduce over
    the 8 cores (path warmed by an early dummy AllReduce; remote_dma is not
    functional in this runtime — it wedges the device); probs = exp * (1/S)
    on-core.
"""

import sys

for _p in ("/opt/trn_rl_repo",):
    if _p not in sys.path:
        sys.path.insert(0, _p)

import numpy as np
import ml_dtypes
from concourse import bass, bacc, mybir, tile
from concourse import bass_utils


# ---------------------------------------------------------------- constants
N_CORES = 8
A_FULL = 100000
N_NODES = 50000
N_REGIONS = 8
D = 128
H = 256
G = 147
IN_DIM = 2 * D + N_REGIONS * D + G          # 1427
TAIL_LEN = N_REGIONS * D + G                # 1171
F32 = mybir.dt.float32
BF16 = mybir.dt.bfloat16
I16 = mybir.dt.int16

A_PC = A_FULL // N_CORES                    # 12500
SPLIT = 32768                               # int16 index range boundary
C0 = 8704                                   # capacity, node id < 32768 (17*512)
C1 = 4608                                   # capacity, node id >= 32768 (9*512)
A_PAD = C0 + C1                             # 13312 = 26*512 = 104*128
N_CHUNKS = A_PAD // 128                     # 104
ATILE = 512
N_AT = A_PAD // ATILE                       # 26
SWEEP = 4
SWSZ = SWEEP * ATILE                        # 2048
N_SW = (N_AT + SWEEP - 1) // SWEEP          # 7 sweeps (last one ragged)

# softmax prep ranges: [slot0, slot1, col0) — ranges 0/1 via HBM reshuffle
# mid-loop, range 2 via PE rank-1 transposes on the tail.
PREP = ((0, 6656, 0), (6656, 12288, 52), (12288, 13312, 96))

EXP_SHIFT = -4.0
N_WARMUP_MM = 190
WARM_COLLECTIVE = True

# pk (f32 per-core constant pack) column layout
PK_B1C = 0            # 0:2   b1c = tail@W1[256:] + b1, [128,2]
PK_B2 = 2             # 2:4
PK_B3 = 4             # 4:6
PK_MASK = 6           # 6:110 softmax mask in [128, N_CHUNKS] prep layout
PK_B4 = 110           # [0,110]
PK_SHIFT = 111        # 111:112 = EXP_SHIFT
PK_ONES = 112         # 112:240 = 1.0 (col 112 as [128,1]; row 0 as [1,128])
PK_COLS = 240

# wpack (bf16 weight pack) column layout
WP_W1A = 0            # 0:256
WP_W2 = 256           # 256:768   (two 256-col blocks)
WP_W3 = 768           # 768:1280
WP_W4 = 1280          # 1280:1282
WP_ID = 1282          # 1282:1410 identity for PE transposes
WP_COLS = 1410


def _gather_calls():
    """Sweep-aligned gather calls: (slot0, n, group, group_local_off).
    The first two are small so the pipeline's first data lands early."""
    calls = [(0, 256, 0, 0), (256, 256, 0, 256), (512, 512, 0, 512)]
    for s0 in range(1024, C0 - 512, 1024):
        calls.append((s0, 1024, 0, s0))
    calls.append((C0 - 512, 512, 0, C0 - 512))
    calls.append((C0, 512, 1, 0))
    for k in range(4):
        calls.append((C0 + 512 + k * 1024, 1024, 1, 512 + k * 1024))
    return calls


def build_graph():
    nc = bacc.Bacc("TRN2", target_bir_lowering=False, debug=False,
                   num_devices=N_CORES, num_swdge_queues=4)

    # ---- I/O --------------------------------------------------------------
    node_emb = nc.dram_tensor("node_emb", [N_NODES, D], BF16, kind="ExternalInput")
    wpack = nc.dram_tensor("wpack", [128, WP_COLS], BF16, kind="ExternalInput")
    rpb = nc.dram_tensor("rpb", [N_REGIONS, H], BF16, kind="ExternalInput")
    pk_in = nc.dram_tensor("packed", [128, PK_COLS], F32, kind="ExternalInput")
    idx0 = nc.dram_tensor("idx0", [128, C0 // 16], I16, kind="ExternalInput")
    idx1 = nc.dram_tensor("idx1", [128, C1 // 16], I16, kind="ExternalInput")
    onehot = nc.dram_tensor("onehot", [N_REGIONS, A_PAD], BF16, kind="ExternalInput")

    out_logits = nc.dram_tensor("out_logits", [1, A_PAD], F32, kind="ExternalOutput")
    out_probs = nc.dram_tensor("out_probs", [128, N_CHUNKS], F32, kind="ExternalOutput")

    calls = _gather_calls()

    with tile.TileContext(nc) as tc:
        with (
            tc.tile_pool(name="const", bufs=1) as cpool,
            tc.tile_pool(name="hbuf", bufs=2) as hpool,
            tc.tile_pool(name="ph", bufs=5, space="PSUM") as ph_pool,
            tc.tile_pool(name="pnt", bufs=1, space="PSUM") as pnt_pool,
            tc.tile_pool(name="plg", bufs=2, space="PSUM") as plg_pool,
            tc.tile_pool(name="dram", bufs=1, space="DRAM") as dpool,
        ):
            # ---- SWDGE gather-ucode warm: dep-free dummy gather first ----
            # (the first dma_gather triggers a ~9us Q7 ucode load; pay it
            # immediately so the real gathers' data lands earlier)
            idummy = cpool.tile([128, 8], I16, tag="idummy")
            nc.gpsimd.memset(idummy[:], 0)
            gdummy = cpool.tile([128, 1, D], BF16, tag="gdummy")
            rdummy = nc.gpsimd.to_reg(128)
            nc.gpsimd.dma_gather(
                out_ap=gdummy[:], in_ap=node_emb[0:SPLIT, :],
                idxs_ap=idummy[:], num_idxs=128, num_idxs_reg=rdummy,
                elem_size=D, transpose=False, single_packet=False,
                queue_num=0)

            # ---- index loads first: gathers depend on them ---------------
            # dedicated tiny tiles for the first two calls so call 0 can
            # dispatch as soon as 4KB lands (not the whole 139KB table)
            i0a = cpool.tile([128, 16], I16, tag="i0a")
            nc.sync.dma_start(out=i0a[:], in_=idx0[:, 0:16])
            i0b = cpool.tile([128, 16], I16, tag="i0b")
            nc.sync.dma_start(out=i0b[:], in_=idx0[:, 16:32])
            i0 = cpool.tile([128, C0 // 16], I16, tag="i0")
            nc.sync.dma_start(out=i0[:], in_=idx0[:])
            i1 = cpool.tile([128, C1 // 16], I16, tag="i1")
            nc.sync.dma_start(out=i1[:], in_=idx1[:])

            # ---- all gathers upfront, round-robin over 4 SWDGE queues ----
            regs = {n: nc.gpsimd.to_reg(n)
                    for n in sorted({c[1] for c in _gather_calls()})}
            graws = []
            for gi, (s0, n, grp, loff) in enumerate(calls):
                gsrc = node_emb[0:SPLIT, :] if grp == 0 \
                    else node_emb[SPLIT:N_NODES, :]
                if gi == 0:
                    iap = i0a[:]
                elif gi == 1:
                    iap = i0b[:]
                else:
                    itile = i0 if grp == 0 else i1
                    iap = itile[:, loff // 16:(loff + n) // 16]
                graw = cpool.tile([128, n // 128, D], BF16, tag=f"graw{gi}")
                nc.gpsimd.dma_gather(
                    out_ap=graw[:],
                    in_ap=gsrc,
                    idxs_ap=iap,
                    num_idxs=n, num_idxs_reg=regs[n],
                    elem_size=D, transpose=False, single_packet=False,
                    queue_num=(gi + 1) % 4)
                graws.append(graw)

            # ---- warm the collectives path with a dummy 4B AllReduce -----
            # (the entry barrier + dummy round complete while the loop runs)
            if WARM_COLLECTIVE:
                ccd_in = dpool.tile([1, 1], F32, name="ccd_in")
                ccd_out = dpool.tile([1, 1], F32, addr_space="Shared",
                                     name="ccd_out")
                nc.scalar.dma_start(out=ccd_in[:], in_=pk_in[0:1, 0:1])
                nc.gpsimd.collective_compute(
                    "AllReduce", mybir.AluOpType.add,
                    replica_groups=[list(range(N_CORES))],
                    ins=[ccd_in.opt()], outs=[ccd_out.opt()])

            # ---- constant loads (all HWDGE on sync) ----------------------
            wp = cpool.tile([128, WP_COLS], BF16, tag="wp")
            nc.sync.dma_start(out=wp[:], in_=wpack[:])
            w1a = wp[:, WP_W1A:WP_W1A + H]
            w2t = [wp[:, WP_W2 + k * H:WP_W2 + (k + 1) * H] for k in range(2)]
            w3t = [wp[:, WP_W3 + k * H:WP_W3 + (k + 1) * H] for k in range(2)]
            w4s = wp[:, WP_W4:WP_W4 + 2]
            ident = wp[:, WP_ID:WP_ID + 128]

            rps = cpool.tile([N_REGIONS, H], BF16, tag="rps")
            nc.sync.dma_start(out=rps[:], in_=rpb[:])
            pk = cpool.tile([128, PK_COLS], F32, tag="pk")
            nc.sync.dma_start(out=pk[:], in_=pk_in[:])
            b1s = pk[:, PK_B1C:PK_B1C + 2]
            b2s = pk[:, PK_B2:PK_B2 + 2]
            b3s = pk[:, PK_B3:PK_B3 + 2]
            masks = pk[:, PK_MASK:PK_MASK + N_CHUNKS]
            b4s = pk[0:1, PK_B4:PK_B4 + 1]
            shift = pk[:, PK_SHIFT:PK_SHIFT + 1]
            ones_c = pk[:, PK_ONES:PK_ONES + 1]
            ones_r = pk[0:1, PK_ONES:PK_ONES + 128]
            one_s = pk[0:1, PK_ONES:PK_ONES + 1]
            ohs = cpool.tile([N_REGIONS, A_PAD], BF16, tag="ohs")
            nc.sync.dma_start(out=ohs[:], in_=onehot[:])

            # ---- PE warmup: spin the HAM clock up while gathers land -----
            if N_WARMUP_MM:
                wm = ph_pool.tile([128, ATILE], F32, space="PSUM",
                                  tag="hps", name="hps")
                for _ in range(N_WARMUP_MM):
                    nc.tensor.matmul(out=wm[:, 0:128], lhsT=ident, rhs=ident,
                                     start=True, stop=True)

            nts_all = cpool.tile([128, A_PAD], BF16, tag="nts_all")
            lrow = cpool.tile([1, A_PAD], F32, tag="lrow")
            lgT = cpool.tile([128, N_CHUNKS], F32, tag="lgT")
            expt = cpool.tile([128, N_CHUNKS], F32, tag="expt")
            em = cpool.tile([128, N_CHUNKS], F32, tag="em")
            srow = cpool.tile([128, 1], F32, tag="srow")

            def evict_relu(engine, dst, src, bias_ap):
                if engine == "act":
                    nc.scalar.activation(
                        out=dst, in_=src,
                        func=mybir.ActivationFunctionType.Relu, bias=bias_ap)
                else:
                    nc.vector.tensor_scalar(
                        out=dst, in0=src, scalar1=bias_ap, scalar2=0.0,
                        op0=mybir.AluOpType.add, op1=mybir.AluOpType.max)

            def transpose_sweep(lo, hi):
                """PE-transpose this range's gathered chunks into nts_all."""
                for gi, (g0, n, grp, loff) in enumerate(calls):
                    if g0 + n <= lo or g0 >= hi:
                        continue
                    nt_ps = pnt_pool.tile([128, 1024], BF16, space="PSUM",
                                          tag="nt_ps", name="nt_ps")
                    for c in range(n // 128):
                        nc.tensor.transpose(
                            out=nt_ps[:, c * 128:(c + 1) * 128],
                            in_=graws[gi][:, c, :], identity=ident)
                    nc.vector.tensor_copy(out=nts_all[:, g0:g0 + n],
                                          in_=nt_ps[:, 0:n])

            def softmax_prep(pi):
                a0, a1, c0 = PREP[pi]
                c1 = c0 + (a1 - a0) // 128
                nc.sync.dma_start(out=out_logits[0:1, a0:a1],
                                  in_=lrow[0:1, a0:a1])
                nc.sync.dma_start(
                    out=lgT[:, c0:c1],
                    in_=out_logits[0:1, a0:a1].rearrange(
                        "o (p t) -> (o p) t", p=128))
                nc.scalar.activation(out=expt[:, c0:c1], in_=lgT[:, c0:c1],
                                     func=mybir.ActivationFunctionType.Exp,
                                     bias=shift, scale=1.0)
                nc.vector.tensor_tensor(
                    out=em[:, c0:c1], in0=expt[:, c0:c1],
                    in1=masks[:, c0:c1], op=mybir.AluOpType.mult)

            # ---- main loop: graded sweeps (small early ones fill the
            # pipeline while the first gathers land) ------------------------
            plan = [(0, 2), (2, 4), (4, 8), (8, 12), (12, 16), (16, 20),
                    (20, 24), (24, 26)]
            for (t0, t1) in plan:
                transpose_sweep(t0 * ATILE, t1 * ATILE)
                tiles = list(range(t0, t1))
                sls = [slice(t * ATILE, (t + 1) * ATILE) for t in tiles]
                nt = len(tiles)

                # layer 1
                h1 = [[hpool.tile([128, ATILE], BF16, tag=f"h1_{j}_{i}",
                                  name=f"h1_{j}_{i}")
                       for j in range(2)] for i in range(nt)]
                for j in range(2):
                    hps = [ph_pool.tile([128, ATILE], F32, space="PSUM",
                                        tag="hps", name="hps")
                           for _ in range(nt)]
                    for i in range(nt):
                        nc.tensor.matmul(out=hps[i][:],
                                         lhsT=w1a[:, j * 128:(j + 1) * 128],
                                         rhs=nts_all[:, sls[i]],
                                         start=True, stop=False)
                    for i in range(nt):
                        nc.tensor.matmul(out=hps[i][:],
                                         lhsT=rps[0:8, j * 128:(j + 1) * 128],
                                         rhs=ohs[0:8, sls[i]],
                                         start=False, stop=True)
                    for i in range(nt):
                        evict_relu("act" if (i + j) % 2 == 0 else "dve",
                                   h1[i][j][:], hps[i][:], b1s[:, j:j + 1])

                # layers 2 and 3
                hin = h1
                for li, (wt, bs) in enumerate(((w2t, b2s), (w3t, b3s))):
                    hout = [[hpool.tile([128, ATILE], BF16,
                                        tag=f"h{li + 2}_{j}_{i}",
                                        name=f"h{li + 2}_{j}_{i}")
                             for j in range(2)] for i in range(nt)]
                    for j in range(2):
                        hps = [ph_pool.tile([128, ATILE], F32, space="PSUM",
                                            tag="hps", name="hps")
                               for _ in range(nt)]
                        for k in range(2):
                            for i in range(nt):
                                nc.tensor.matmul(
                                    out=hps[i][:],
                                    lhsT=wt[k][:, j * 128:(j + 1) * 128],
                                    rhs=hin[i][k][:],
                                    start=(k == 0), stop=(k == 1))
                        for i in range(nt):
                            evict_relu("act" if (i + j + li) % 2 == 0 else "dve",
                                       hout[i][j][:], hps[i][:], bs[:, j:j + 1])
                    hin = hout

                # layer 4: logits
                for i in range(nt):
                    lg = plg_pool.tile([1, ATILE], F32, space="PSUM", tag="lg")
                    for k in range(2):
                        nc.tensor.matmul(out=lg[:], lhsT=w4s[:, k:k + 1],
                                         rhs=hin[i][k][:],
                                         start=(k == 0), stop=(k == 1))
                    nc.scalar.activation(
                        out=lrow[0:1, sls[i]], in_=lg[:],
                        func=mybir.ActivationFunctionType.Identity,
                        bias=b4s)

                if tiles[-1] == 15:                 # slots 0:6656 done
                    softmax_prep(0)
                elif tiles[-1] == 23:               # slots 6656:12288 done
                    softmax_prep(1)

            # ---- tail: last 1024 logits via PE rank-1 transposes ---------
            a0, a1, c0 = PREP[2]
            lgps = plg_pool.tile([128, 8], F32, space="PSUM", tag="lg")
            for b in range(8):
                nc.tensor.matmul(out=lgps[:, b:b + 1],
                                 lhsT=lrow[0:1, a0 + b * 128:a0 + (b + 1) * 128],
                                 rhs=one_s, start=True, stop=True)
            nc.scalar.activation(out=expt[:, c0:c0 + 8], in_=lgps[:],
                                 func=mybir.ActivationFunctionType.Exp,
                                 bias=shift, scale=1.0)
            nc.vector.tensor_tensor(out=em[:, c0:c0 + 8],
                                    in0=expt[:, c0:c0 + 8],
                                    in1=masks[:, c0:c0 + 8],
                                    op=mybir.AluOpType.mult)
            nc.vector.tensor_reduce(out=srow[:], in_=em[:],
                                    axis=mybir.AxisListType.X,
                                    op=mybir.AluOpType.add)
            # partition-sum via PE: [1,1] = ones.T @ srow
            s_ps = plg_pool.tile([1, 1], F32, space="PSUM", tag="lg")
            nc.tensor.matmul(out=s_ps[:], lhsT=ones_c, rhs=srow[:],
                             start=True, stop=True)
            s_sb = cpool.tile([1, 1], F32, tag="s_sb")
            nc.scalar.activation(out=s_sb[:], in_=s_ps[:],
                                 func=mybir.ActivationFunctionType.Copy)

            # ---- 4-byte AllReduce over the 8 cores ----------------------
            cc_in = dpool.tile([1, 1], F32, name="cc_in")
            cc_out = dpool.tile([1, 1], F32, addr_space="Shared", name="cc_out")
            nc.scalar.dma_start(out=cc_in[:], in_=s_sb[:])
            nc.gpsimd.collective_compute(
                "AllReduce", mybir.AluOpType.add,
                replica_groups=[list(range(N_CORES))],
                ins=[cc_in.opt()], outs=[cc_out.opt()])
            # store the tail logits while the collective runs
            nc.sync.dma_start(out=out_logits[0:1, a0:a1],
                              in_=lrow[0:1, a0:a1])
            sg = cpool.tile([1, 1], F32, tag="sg")
            nc.scalar.dma_start(out=sg[:], in_=cc_out[:])

            # reciprocal then partition-broadcast via PE: [128,1] = ones @ rb
            rb = cpool.tile([1, 1], F32, tag="rb")
            nc.vector.reciprocal(out=rb[:], in_=sg[:])
            rb_ps = plg_pool.tile([128, 1], F32, space="PSUM", tag="lg")
            nc.tensor.matmul(out=rb_ps[:], lhsT=ones_r,
                             rhs=rb[:], start=True, stop=True)
            rbb = cpool.tile([128, 1], F32, tag="rbb")
            nc.scalar.activation(out=rbb[:], in_=rb_ps[:],
                                 func=mybir.ActivationFunctionType.Copy)

            probs = cpool.tile([128, N_CHUNKS], F32, tag="probs")
            nc.vector.tensor_scalar_mul(out=probs[:], in0=em[:], scalar1=rbb[:])
            nc.sync.dma_start(out=out_probs[:], in_=probs[:])

    nc.compile()
    return nc


_GRAPH_CACHE = {}


def _get_graph():
    if "g" not in _GRAPH_CACHE:
        _GRAPH_CACHE["g"] = build_graph()
    return _GRAPH_CACHE["g"]


def _wrap_idx(ix):
    """int16 index layout for dma_gather: [16, N/16] column-wrapped,
    replicated 8x down the partitions."""
    w = ix.reshape(-1, 16).T
    return np.ascontiguousarray(np.tile(w, (8, 1)))


def _slot_to_pc(slots):
    """Padded slot -> (prow, pcol) of the [128, N_CHUNKS] prep layout."""
    prow = np.empty(len(slots), np.int64)
    pcol = np.empty(len(slots), np.int64)
    for pi, (a0, a1, c0) in enumerate(PREP):
        m = (slots >= a0) & (slots < a1)
        r = slots[m] - a0
        if pi < 2:
            t = (a1 - a0) // 128
            prow[m] = r // t
            pcol[m] = c0 + r % t
        else:
            prow[m] = r % 128
            pcol[m] = c0 + r // 128
    return prow, pcol


def make_in_maps(node_embeddings, region_embeddings, global_context,
                 W1, b1, W2, b2, W3, b3, W4, b4,
                 action_nodes, action_regions):
    """Host-side sharding / marshalling. Returns (in_maps, per-core metas)."""
    W1 = np.asarray(W1, dtype=np.float32)
    an = np.asarray(action_nodes).astype(np.int64)
    ar = np.asarray(action_regions).astype(np.int64)
    node_bf16 = np.ascontiguousarray(
        np.asarray(node_embeddings, np.float32).astype(ml_dtypes.bfloat16))

    # constant folding (host): b1c = tail @ W1[256:] + b1,
    # RP = region_emb @ W1[128:256]
    tail = np.concatenate([
        np.asarray(region_embeddings, np.float32).reshape(-1),
        np.asarray(global_context, np.float32).reshape(-1)])
    b1c = tail @ W1[2 * D:, :] + np.asarray(b1, np.float32)     # [256]
    rp = (np.asarray(region_embeddings, np.float32)
          @ W1[D:2 * D, :]).astype(ml_dtypes.bfloat16)          # [8, 256]

    # bf16 weight pack
    wpack = np.zeros((128, WP_COLS), ml_dtypes.bfloat16)
    wpack[:, WP_W1A:WP_W1A + H] = W1[0:D, :].astype(ml_dtypes.bfloat16)
    W2 = np.asarray(W2, np.float32)
    W3 = np.asarray(W3, np.float32)
    for k in range(2):
        wpack[:, WP_W2 + k * H:WP_W2 + (k + 1) * H] = \
            W2[k * 128:(k + 1) * 128, :].astype(ml_dtypes.bfloat16)
        wpack[:, WP_W3 + k * H:WP_W3 + (k + 1) * H] = \
            W3[k * 128:(k + 1) * 128, :].astype(ml_dtypes.bfloat16)
    wpack[:, WP_W4:WP_W4 + 2] = np.asarray(W4, np.float32).reshape(
        2, 128).T.astype(ml_dtypes.bfloat16)
    wpack[:, WP_ID:WP_ID + 128] = np.eye(128, dtype=ml_dtypes.bfloat16)

    pk_base = np.zeros((128, PK_COLS), np.float32)
    pk_base[:, PK_B1C:PK_B1C + 2] = b1c.reshape(2, 128).T
    pk_base[:, PK_B2:PK_B2 + 2] = np.asarray(b2, np.float32).reshape(2, 128).T
    pk_base[:, PK_B3:PK_B3 + 2] = np.asarray(b3, np.float32).reshape(2, 128).T
    pk_base[0, PK_B4] = np.asarray(b4, np.float32).reshape(-1)[0]
    pk_base[:, PK_SHIFT] = EXP_SHIFT
    pk_base[:, PK_ONES:PK_ONES + 128] = 1.0

    in_maps, metas = [], []
    for c in range(N_CORES):
        s = c * A_PC
        nodes = an[s:s + A_PC]
        regions = ar[s:s + A_PC]
        grp = (nodes >= SPLIT).astype(np.int8)
        order = np.argsort(grp, kind="stable")      # group0 first, stable
        c0 = int((grp == 0).sum())
        c1 = A_PC - c0
        if c0 > C0 or c1 > C1:
            raise RuntimeError(
                f"core {c}: group sizes {c0}/{c1} exceed capacities {C0}/{C1}")
        sn = nodes[order]
        sr = regions[order]

        ix0 = np.zeros(C0, np.int16)
        ix0[:c0] = sn[:c0].astype(np.int16)
        ix1 = np.zeros(C1, np.int16)
        ix1[:c1] = (sn[c0:] - SPLIT).astype(np.int16)

        slots = np.concatenate([np.arange(c0), C0 + np.arange(c1)])
        oh = np.zeros((N_REGIONS, A_PAD), ml_dtypes.bfloat16)
        oh[sr, slots] = 1.0

        prow, pcol = _slot_to_pc(slots)
        mask = np.zeros((128, N_CHUNKS), np.float32)
        mask[prow, pcol] = 1.0

        pkc = pk_base.copy()
        pkc[:, PK_MASK:PK_MASK + N_CHUNKS] = mask
        in_maps.append({
            "node_emb": node_bf16,
            "wpack": wpack, "rpb": rp, "packed": pkc,
            "idx0": _wrap_idx(ix0), "idx1": _wrap_idx(ix1),
            "onehot": oh,
        })
        metas.append((order, slots, prow, pcol))
    return in_maps, metas


def assemble(per_core_outs, metas):
    """Un-shard per-core {out_logits, out_probs} into full (probs, logits)."""
    probs = np.empty(A_FULL, np.float32)
    logits = np.empty(A_FULL, np.float32)
    for c in range(N_CORES):
        order, slots, prow, pcol = metas[c]
        out = per_core_outs[c]
        lg = np.asarray(out["out_logits"]).reshape(-1)[slots]
        pb = np.asarray(out["out_probs"]).reshape(128, N_CHUNKS)[prow, pcol]
        logits[c * A_PC:(c + 1) * A_PC][order] = lg
        probs[c * A_PC:(c + 1) * A_PC][order] = pb
    return probs, logits


def kernel(**inputs):
    nc = _get_graph()
    in_maps, metas = make_in_maps(**inputs)
    res = bass_utils.run_bass_kernel_spmd(
        nc, in_maps, core_ids=list(range(N_CORES)))
    return assemble(res.results, metas)
